# revision 1
# baseline (speedup 1.0000x reference)
"""LengthRegulator kernel for Trainium2 (Bass/Tile), 8-core data parallel.

Reference op, per batch row b:
    dur  = clamp(durations[b].astype(int32), min=0)          # [T]
    csum = cumsum(dur)                                       # [T] inclusive
    src[j] = searchsorted(csum, j, 'right') = #{t: csum[t] <= j}   j in [0, L)
    out[b, j] = x[b, src[j]] if j < csum[-1] else 0

Device algorithm (per row; B=16 rows, 2 per core), bf16 data path:
    - x is converted to bf16 host-side and padded with a zero row at index T;
      a row-gather with the *unclamped* src (== T exactly when j >= total)
      produces the masked output directly. Output is stored bf16 and
      converted back to f32 host-side (max rel err ~2^-9, far under 2e-2).
      bf16 halves the bytes through the serialized DMA-engines device, which
      is the kernel's roofline: gather(8MiB) + store(8MiB) per core.
    - cumsum: per-partition scan (tensor_tensor_scan) + strict-triangle
      matmul for cross-partition offsets; both rows' csum computed up front.
    - searchsorted: src[j] = sum_{p,f} (csum[p,f] <= J[j]) via 8 DVE compares
      [128, 2048] against per-partition scalars, reduced over partitions with
      ones-vector matmuls accumulating in PSUM.
    - J column order is k-major so each 2048-row output chunk's indices
      complete independently: col = 2048k + 128w + 8m + r holds j-value
      2048k + 256r + 16w + m (validated vs numpy reference). Chunk k's
      gather can launch as soon as half k's compares/matmuls finish, so the
      first gather starts ~13us in instead of ~40us.
    - gather chunk k: idx16_k[16g + w, f] = R[32k, 128w + f] (8 small DMAs
      replicate the 16-partition index block across the 8 SWDGE channel
      groups). dma_gather pulls 1KiB bf16 rows from HBM; dst[i%128, i//128] =
      x_pad[idx#i] with idx#i = idx16_k[i%16, i//16], which makes the store
      DMA 16KiB-contiguous per (partition, chunk) descriptor.
"""

import numpy as np

B, T, D, L = 16, 1024, 512, 4096
NCORES = 8
RPC = B // NCORES  # batch rows per core

_cache = {}


def _build_nc(reps=1):
    import concourse.bacc as bacc
    import concourse.mybir as mybir
    import concourse.tile as tile
    from concourse import library_config

    f32 = mybir.dt.float32
    bf16 = mybir.dt.bfloat16
    i32 = mybir.dt.int32
    i16 = mybir.dt.int16
    Alu = mybir.AluOpType

    nc = bacc.Bacc("TRN2", target_bir_lowering=False)
    x_pad = nc.dram_tensor("x_pad", [RPC, T + 1, D], bf16, kind="ExternalInput")
    dur_in = nc.dram_tensor("dur", [RPC, T], i32, kind="ExternalInput")
    out = nc.dram_tensor("out", [RPC, L, D], bf16, kind="ExternalOutput")

    # J constant, k-major: col = 2048k + 128w + 8m + r -> j = 2048k + 256r
    # + 16w + m (validated vs numpy reference; see module docstring).
    kk_, w_, m_, r_ = np.meshgrid(
        np.arange(2), np.arange(16), np.arange(16), np.arange(8), indexing="ij"
    )
    J_host = (2048 * kk_ + 256 * r_ + 16 * w_ + m_).reshape(-1)
    J16_const = nc.inline_tensor(
        np.broadcast_to(J_host, (128, L)).astype(np.int16), name="J16_const"
    )
    U_const = nc.inline_tensor(
        np.triu(np.ones((128, 128), np.float32), k=1), name="U_const"
    )

    with tile.TileContext(nc) as tc:
        with (
            tc.tile_pool(name="const", bufs=1) as cpool,
            tc.tile_pool(name="small", bufs=2) as spool,
            tc.tile_pool(name="idx", bufs=3) as ipool,
            tc.tile_pool(name="cmp", bufs=2) as cmppool,
            tc.tile_pool(name="gath", bufs=3) as gpool,
            tc.tile_pool(name="psmall", bufs=1, space="PSUM") as ppool,
            tc.tile_pool(name="pR", bufs=1, space="PSUM") as rpool,
        ):
            # ---- constants / inputs ----
            nc.gpsimd.load_library(library_config.mlp)
            dur_t = spool.tile([128, 2 * 8], i32, tag="dur")
            nc.scalar.dma_start(
                out=dur_t[:].rearrange("p (r f) -> p r f", r=RPC),
                in_=dur_in[:].rearrange("r (p f) -> p r f", p=128),
            )
            U = cpool.tile([128, 128], f32)  # U[k, m] = 1 iff k < m
            nc.sync.dma_start(out=U[:], in_=U_const[:])
            J16 = cpool.tile([128, L], i16)
            nc.sync.dma_start(out=J16[:, 0:2048], in_=J16_const[:, 0:2048])
            nc.scalar.dma_start(out=J16[:, 2048:4096], in_=J16_const[:, 2048:4096])
            ones = cpool.tile([128, 1], bf16)
            nc.vector.memset(ones[:], 1.0)

            # ---- cumsum of clamped durations, both rows up front ----
            dur_f = spool.tile([128, 2 * 8], f32, tag="durf")
            nc.vector.tensor_scalar(dur_f[:], dur_t[:], 0, None, Alu.max)
            pref = spool.tile([128, 2 * 8], f32, tag="pref")
            csum = spool.tile([128, 2 * 8], f32, tag="csum")
            offs = ppool.tile([128, RPC], f32, tag="offs")
            for r in range(RPC):
                sl = slice(8 * r, 8 * r + 8)
                nc.vector.tensor_tensor_scan(
                    out=pref[:, sl],
                    data0=dur_f[:, sl],
                    data1=dur_f[:, sl],
                    initial=0.0,
                    op0=Alu.add,
                    op1=Alu.bypass,
                )
                nc.tensor.matmul(
                    out=offs[:, r : r + 1],
                    lhsT=U[:],
                    rhs=pref[:, 8 * r + 7 : 8 * r + 8],
                    start=True,
                    stop=True,
                )
                nc.vector.tensor_tensor(
                    out=csum[:, sl],
                    in0=pref[:, sl],
                    in1=offs[:, r : r + 1].to_broadcast([128, 8]),
                    op=Alu.add,
                )

            # ---- per (row, chunk): searchsorted + gather + store ----
            for rep in range(reps):
                for r in range(RPC):
                    R = rpool.tile([33, 2048], f32, tag="R")
                    for k in range(2):
                        jsl = slice(2048 * k, 2048 * (k + 1))
                        for f in range(8):
                            C = cmppool.tile([128, 2048], bf16, tag="C")
                            nc.vector.tensor_scalar(
                                C[:],
                                J16[:, jsl],
                                csum[:, 8 * r + f : 8 * r + f + 1],
                                None,
                                Alu.is_ge,
                            )
                            for c in range(4):
                                nc.tensor.matmul(
                                    out=R[32 * k : 32 * k + 1, 512 * c : 512 * (c + 1)],
                                    lhsT=ones[:],
                                    rhs=C[:, 512 * c : 512 * (c + 1)],
                                    start=(f == 0),
                                    stop=(f == 7),
                                )
                        idx_row = spool.tile([33, 2048], i16, tag="idxrow")
                        nc.vector.tensor_copy(
                            out=idx_row[32 * k : 32 * k + 1, :],
                            in_=R[32 * k : 32 * k + 1, :],
                        )
                        idx16 = ipool.tile([128, 128], i16, tag="idx16")
                        for g in range(8):
                            eng = nc.sync if g % 2 == 0 else nc.scalar
                            eng.dma_start(
                                out=idx16[16 * g : 16 * g + 16, :],
                                in_=idx_row[32 * k : 32 * k + 1, :],
                            )
                        gt = gpool.tile([128, 16 * D], bf16, tag="gt")
                        nc.gpsimd.dma_gather(
                            out_ap=gt[:].rearrange("p (m e) -> p m e", e=D),
                            in_ap=x_pad[r],
                            idxs_ap=idx16[:],
                            num_idxs=2048,
                            num_idxs_reg=2048,
                            elem_size=D,
                            single_packet=False,
                        )
                        eng = nc.sync if k == 0 else nc.scalar
                        eng.dma_start(
                            out=out[r, 2048 * k : 2048 * (k + 1), :].rearrange(
                                "(p m) e -> p m e", p=128
                            ),
                            in_=gt[:].rearrange("p (m e) -> p m e", e=D),
                        )
    nc.compile()
    return nc


def _get_nc(reps=1):
    if reps not in _cache:
        _cache[reps] = _build_nc(reps)
    return _cache[reps]


def kernel(x, durations, max_len):
    import ml_dtypes
    from concourse.bass_utils import run_bass_kernel_spmd

    x = np.asarray(x)
    durations = np.asarray(durations)
    assert x.shape == (B, T, D) and int(max_len) == L, (x.shape, max_len)

    dur32 = durations.astype(np.int32)  # truncating cast, same as reference
    in_maps = []
    for core in range(NCORES):
        lo = core * RPC
        xp = np.zeros((RPC, T + 1, D), ml_dtypes.bfloat16)
        xp[:, :T, :] = x[lo : lo + RPC].astype(ml_dtypes.bfloat16)
        in_maps.append({"x_pad": xp, "dur": np.ascontiguousarray(dur32[lo : lo + RPC])})

    nc = _get_nc()
    res = run_bass_kernel_spmd(nc, in_maps, core_ids=list(range(NCORES)))
    outs = [np.asarray(res.results[c]["out"]).astype(np.float32) for c in range(NCORES)]
    return np.concatenate(outs, axis=0).reshape(B, L, D)



# revision 8
# speedup vs baseline: 1.2729x; 1.2729x over previous
"""LengthRegulator kernel for Trainium2 (Bass/Tile), 8-core data parallel.

Reference op, per batch row b:
    dur  = clamp(durations[b].astype(int32), min=0)               # [T]
    csum = cumsum(dur)                                            # [T] incl
    src[j] = searchsorted(csum, j, 'right')          j in [0, L)
    out[b, j] = x[b, src[j]] if j < csum[-1] else 0

Device algorithm (B=16 rows, 2 per core), bf16 data path:

  searchsorted via sorted-rank max-scan (NOT a counting histogram --
  dma_scatter_add duplicate indices race on HW and lose adds):
    csum is non-decreasing, so src[j] = max{t+1 : csum[t] <= j}.  For each
    "last token of its equal-csum group" (mask = dur[t+1] > 0, sentinel at
    t=T-1) scatter the VALUE t+1 into bin csum[t] -- bins are unique, so
    the scatter-add never races.  Bins are SBUF-parity-scatter encoded as
    sigma = (csum%256)*128 + csum//256: partition = csum//256 (<16 for
    valid j), slot = csum%256 -> parity tiles h_even/h_odd [128,128] f32.
    Tokens with csum >= 4096 land on partitions 16..28 (never read) --
    automatic dump; masked tokens are directed to partition 16.
    src[j] then = max(carry[j//256], running-max over slots of A0) where
    A0[q, s] = h[256q + s] (two strided DVE copies from h_even/h_odd) and
    carry[q] = #{t: csum[t] < 256q} (sorted => count == max-rank), computed
    off the critical path as is_ge-compare [16,16,64] -> X-reduce ->
    ones-matmul into PSUM [16,1].

  gather: idx block[a, b'] = src[16b' + a] == the i16 convert of the scan
    output directly (src16 [16,256] covers both 2048-row chunks), then
    3 partition-doubling DMAs replicate to the 8 SWDGE channel groups.
    dma_gather pulls 1KiB bf16 rows from x_pad (row T = zeros serves
    j >= total and src==T); gathered row i of chunk k is out row
    j = 2048k + 128*(i//128) + i%128.

  store: paged_writeback (V path, ncn=128, page_size=128, page-aligned
    static iota indices) scatter-writes the gather tile straight to the
    paged DRAM output -- modeled at ~257 descriptors per 2MiB chunk vs
    2048 for a plain DMA store, removing ~23us of serialized DMA time.
    Host-side de-paging: pages[r, pg, part, 1, :] -> out row 128*pg+part
    (pure layout slice, validated on HW).
"""

import numpy as np

B, T, D, L = 16, 1024, 512, 4096
NCORES = 8
RPC = B // NCORES  # batch rows per core
NPG = L // 128  # writeback pages per row (page_size=128)

_cache = {}


def _consts():
    # U16s[k, m] = 1 iff k < m  (strict upper: offs[m] = sum_{k<m} last[k])
    u16 = np.triu(np.ones((16, 16), np.float32), k=1)
    # thr[q, 64*p + f] = 256*p - 1 ; C = (thr >= csum) <=> csum < 256p
    thr = np.repeat((256.0 * np.arange(16) - 1.0), 64).astype(np.float32)
    thr = np.broadcast_to(thr, (16, 1024)).copy()
    # scatter value layout: desc i reads vals[i%128, i//128] and idx
    # block[i%16, i//16]; block[a, b] = sigma(t = 64a + b)  =>
    # vals[p, s] = t+1 with t = 64*(p%16) + 8*s + p//16
    p = np.arange(128)[:, None]
    s = np.arange(8)[None, :]
    vals = (64 * (p % 16) + 8 * s + p // 16 + 1).astype(np.float32)
    return u16, thr, vals


def _build_nc(reps=1):
    import concourse.bacc as bacc
    import concourse.mybir as mybir
    import concourse.tile as tile
    from concourse import library_config
    from bass_rust import add_dep_helper

    f32 = mybir.dt.float32
    bf16 = mybir.dt.bfloat16
    i32 = mybir.dt.int32
    i16 = mybir.dt.int16
    Alu = mybir.AluOpType

    nc = bacc.Bacc("TRN2", target_bir_lowering=False)
    x_pad = nc.dram_tensor("x_pad", [RPC, T + 1, D], bf16, kind="ExternalInput")
    dur_in = nc.dram_tensor("dur", [RPC, T], i32, kind="ExternalInput")
    durn_in = nc.dram_tensor("durn", [RPC, T], i32, kind="ExternalInput")
    pages = nc.dram_tensor(
        "pages", [RPC, NPG, 128, 2 * D], bf16, kind="ExternalOutput"
    )

    u16_h, thr_h, vals_h = _consts()
    U16c = nc.inline_tensor(u16_h, name="U16c")
    THRc = nc.inline_tensor(thr_h, name="THRc")
    VALc = nc.inline_tensor(vals_h, name="VALc")

    with tile.TileContext(nc) as tc:
        with (
            tc.tile_pool(name="const", bufs=1) as cpool,
            tc.tile_pool(name="small", bufs=2) as spool,
            tc.tile_pool(name="hist", bufs=2) as hpool,
            tc.tile_pool(name="idx", bufs=2) as ipool,
            tc.tile_pool(name="gath", bufs=3) as gpool,
            tc.tile_pool(name="ps", bufs=1, space="PSUM") as ppool,
        ):
            nc.gpsimd.load_library(library_config.attnmlp)

            # ---- constants ----
            U16 = cpool.tile([16, 16], f32)
            nc.sync.dma_start(out=U16[:], in_=U16c[:])
            THR = cpool.tile([16, 1024], f32)
            nc.sync.dma_start(out=THR[:], in_=THRc[:])
            VAL = cpool.tile([128, 8], f32)
            nc.scalar.dma_start(out=VAL[:], in_=VALc[:])
            ones16 = cpool.tile([16, 1], f32)
            nc.vector.memset(ones16[:], 1.0)
            # writeback idx consts: [ptr1(16) | ptr2(16) | pidx(16)] per chunk
            wbi = [
                cpool.tile([128, 48], i32, name=f"wbi{k}") for k in range(2)
            ]
            for k in range(2):
                nc.gpsimd.iota(
                    wbi[k][:, 0:16], pattern=[[1, 16]], base=16 * k,
                    channel_multiplier=0,
                )
                nc.vector.memset(wbi[k][:, 16:32], -1)
                nc.vector.memset(wbi[k][:, 32:48], 0)

            # ---- inputs: durations (t = 64p + f) ----
            dur_t = spool.tile([16, RPC * 64], i32, tag="dur")
            nc.sync.dma_start(
                out=dur_t[:].rearrange("p (r f) -> p r f", r=RPC),
                in_=dur_in[:].rearrange("r (p f) -> p r f", p=16),
            )
            durn_t = spool.tile([16, RPC * 64], i32, tag="durn")
            nc.scalar.dma_start(
                out=durn_t[:].rearrange("p (r f) -> p r f", r=RPC),
                in_=durn_in[:].rearrange("r (p f) -> p r f", p=16),
            )
            dur_f = spool.tile([16, RPC * 64], f32, tag="durf")
            nc.vector.tensor_scalar(dur_f[:], dur_t[:], 0, None, Alu.max)
            m_i = spool.tile([16, RPC * 64], i32, tag="mi")
            nc.vector.tensor_scalar(m_i[:], durn_t[:], 0, None, Alu.is_gt)

            # ---- phase A per row: csum -> sigma -> scatter ----
            csums, scs, carrs = [], [], []
            for r in range(RPC):
                sl = slice(64 * r, 64 * r + 64)
                pref = spool.tile([16, 64], f32, tag=f"pref{r}")
                nc.vector.tensor_tensor_scan(
                    out=pref[:], data0=dur_f[:, sl], data1=dur_f[:, sl],
                    initial=0.0, op0=Alu.add, op1=Alu.bypass,
                )
                offs = ppool.tile([16, 1], f32, tag=f"offs{r}")
                nc.tensor.matmul(
                    out=offs[:], lhsT=U16[:], rhs=pref[:, 63:64],
                    start=True, stop=True,
                )
                csum = spool.tile([16, 64], f32, tag=f"csum{r}")
                nc.vector.tensor_tensor(
                    out=csum[:], in0=pref[:],
                    in1=offs[:].to_broadcast([16, 64]), op=Alu.add,
                )
                csums.append(csum)

                # sigma = (e&255)<<7 | e>>8, masked (non-last -> 16)
                e_i = spool.tile([16, 64], i32, tag=f"ei{r}")
                nc.vector.tensor_copy(out=e_i[:], in_=csum[:])
                sa = spool.tile([16, 64], i32, tag=f"sa{r}")
                nc.vector.tensor_scalar(
                    sa[:], e_i[:], 7, 32640, Alu.logical_shift_left,
                    Alu.bitwise_and,
                )
                hi = spool.tile([16, 64], i32, tag=f"hi{r}")
                nc.vector.tensor_scalar(
                    hi[:], e_i[:], 8, None, Alu.logical_shift_right
                )
                sg = spool.tile([16, 64], i32, tag=f"sg{r}")
                nc.vector.tensor_tensor(
                    out=sg[:], in0=sa[:], in1=hi[:], op=Alu.add
                )
                # select: sigma = m*(sg-16) + 16
                nc.vector.tensor_scalar(sg[:], sg[:], -16, None, Alu.add)
                nc.vector.tensor_tensor(
                    out=sg[:], in0=sg[:], in1=m_i[:, sl], op=Alu.mult
                )
                nc.vector.tensor_scalar(sg[:], sg[:], 16, None, Alu.add)
                sc = ipool.tile([128, 64], i16, tag=f"sc{r}")
                nc.vector.tensor_copy(out=sc[0:16, :], in_=sg[:])
                eng = nc.sync if r == 0 else nc.scalar
                eng.dma_start(out=sc[16:32, :], in_=sc[0:16, :])
                eng.dma_start(out=sc[32:64, :], in_=sc[0:32, :])
                eng.dma_start(out=sc[64:128, :], in_=sc[0:64, :])
                scs.append(sc)

                # carry[q] = #{t: csum < 256q}: compare -> reduce -> matmul
                C = spool.tile([16, 1024], f32, tag=f"C{r}")
                nc.vector.tensor_tensor(
                    out=C[:].rearrange("q (p f) -> q p f", f=64),
                    in0=THR[:].rearrange("q (p f) -> q p f", f=64),
                    in1=csum[:].unsqueeze(1).to_broadcast([16, 16, 64]),
                    op=Alu.is_ge,
                )
                Dm = spool.tile([16, 16], f32, tag=f"D{r}")
                nc.vector.tensor_reduce(
                    out=Dm[:],
                    in_=C[:].rearrange("q (p f) -> q p f", f=64),
                    axis=mybir.AxisListType.X, op=Alu.add,
                )
                carr = ppool.tile([16, 1], f32, tag=f"carr{r}")
                nc.tensor.matmul(
                    out=carr[:], lhsT=Dm[:], rhs=ones16[:],
                    start=True, stop=True,
                )
                carrs.append(carr)

            # scatters back-to-back on Pool so row1's can fire early
            hes, hos = [], []
            for r in range(RPC):
                h_e = hpool.tile([128, 128], f32, tag=f"he{r}")
                h_o = hpool.tile([128, 128], f32, tag=f"ho{r}")
                nc.vector.memset(h_e[:], 0.0)
                nc.vector.memset(h_o[:], 0.0)
                nc.gpsimd.dma_scatter_add(
                    out_ap=h_e[:],
                    in_ap=VAL[:].rearrange("p (s e) -> p s e", e=1),
                    idxs_ap=scs[r][:],
                    num_idxs=1024,
                    num_idxs_reg=1024,
                    elem_size=1,
                    single_packet=False,
                    sbuf_tokens_per_rank=128,
                    parity_reg=0,
                    out_ap_other=h_o[:],
                )
                hes.append(h_e)
                hos.append(h_o)

            # ---- phase B/C per row: scan -> src16 -> gather -> writeback
            for rep in range(reps):
                for r in range(RPC):
                    A0 = spool.tile([16, 256], f32, tag=f"A0{r}")
                    av = A0[:].rearrange("q (s two) -> q s two", two=2)
                    nc.vector.tensor_copy(
                        out=av[:, :, 0:1],
                        in_=hes[r][0:16, :].unsqueeze(2),
                    )
                    nc.vector.tensor_copy(
                        out=av[:, :, 1:2],
                        in_=hos[r][0:16, :].unsqueeze(2),
                    )
                    M = spool.tile([16, 256], f32, tag=f"M{r}")
                    nc.vector.tensor_tensor_scan(
                        out=M[:], data0=A0[:], data1=A0[:], initial=0.0,
                        op0=Alu.max, op1=Alu.bypass,
                    )
                    srcf = spool.tile([16, 256], f32, tag=f"srcf{r}")
                    nc.vector.tensor_tensor(
                        out=srcf[:], in0=M[:],
                        in1=carrs[r][:].to_broadcast([16, 256]), op=Alu.max,
                    )
                    gis = []
                    for k in range(2):
                        # block[a, b] = srcf[a, 128k+b] = src[256a + 128k + b]
                        # (host de-paging applies the inverse permutation)
                        gik = ipool.tile([128, 128], i16, tag=f"gi{r}{k}")
                        nc.vector.tensor_copy(
                            out=gik[0:16, :],
                            in_=srcf[:, 128 * k : 128 * k + 128],
                        )
                        eng = nc.sync if (r + k) % 2 == 0 else nc.scalar
                        eng.dma_start(out=gik[16:32, :], in_=gik[0:16, :])
                        eng.dma_start(out=gik[32:64, :], in_=gik[0:32, :])
                        eng.dma_start(out=gik[64:128, :], in_=gik[0:64, :])
                        gis.append(gik)

                    for k in range(2):
                        gt = gpool.tile([128, 16 * D], bf16, tag="gt")
                        nc.gpsimd.dma_gather(
                            out_ap=gt[:].rearrange("p (m e) -> p m e", e=D),
                            in_ap=x_pad[r],
                            idxs_ap=gis[k][:],
                            num_idxs=2048,
                            num_idxs_reg=2048,
                            elem_size=D,
                            single_packet=False,
                        )
                        wb = nc.gpsimd.paged_writeback(
                            out_ap=pages[r],
                            in_ap=gt[:].rearrange("p (m e) -> p m e", e=D),
                            idxs_ap=wbi[k][:],
                            batch=16,
                            ncn=128,
                            page_size=128,
                            d_head=D,
                            k_or_v="v",
                        )
                        fence = nc.gpsimd.nop(
                            nofuse=True, hint=f"wbf{r}{k}"
                        )
                        add_dep_helper(
                            fence.ins, wb.ins,
                            reason="kernel end waits writeback",
                        )
    nc.compile()
    return nc


def _get_nc(reps=1):
    if reps not in _cache:
        _cache[reps] = _build_nc(reps)
    return _cache[reps]


def kernel(x, durations, max_len):
    import ml_dtypes
    from concourse.bass_utils import run_bass_kernel_spmd

    x = np.asarray(x)
    durations = np.asarray(durations)
    assert x.shape == (B, T, D) and int(max_len) == L, (x.shape, max_len)

    dur32 = durations.astype(np.int32)  # truncating cast, same as reference
    # shifted durations for the "last of equal-csum group" mask; sentinel
    # makes t = T-1 always last (and always a coarse-group end).
    durn32 = np.concatenate(
        [dur32[:, 1:], np.full((B, 1), 8192, np.int32)], axis=1
    )
    in_maps = []
    for core in range(NCORES):
        lo = core * RPC
        xp = np.zeros((RPC, T + 1, D), ml_dtypes.bfloat16)
        xp[:, :T, :] = x[lo : lo + RPC].astype(ml_dtypes.bfloat16)
        in_maps.append(
            {
                "x_pad": xp,
                "dur": np.ascontiguousarray(dur32[lo : lo + RPC]),
                "durn": np.ascontiguousarray(durn32[lo : lo + RPC]),
            }
        )

    nc = _get_nc()
    res = run_bass_kernel_spmd(nc, in_maps, core_ids=list(range(NCORES)))
    # de-page permutation: gathered item i of chunk k holds out row
    # j = 256*(i%16) + 128k + i//16; it lands in page 16k + (i//128) at
    # position u = i%128. Inverting: for out row j,
    #   k = (j//128) % 2, b = (j%128)//8, u = 16*(j%8) + j//256.
    jj = np.arange(L)
    pgi = 16 * ((jj // 128) % 2) + (jj % 128) // 8
    ui = 16 * (jj % 8) + jj // 256
    outs = []
    for c in range(NCORES):
        pg = np.asarray(res.results[c]["pages"])  # [RPC, NPG, 128, 1024]
        pg5 = pg.reshape(RPC, NPG, 128, 2, D)
        rows = pg5[:, pgi, ui, 1, :]  # [RPC, L, D]
        outs.append(rows.astype(np.float32))
    return np.concatenate(outs, axis=0).reshape(B, L, D)


# revision 11
# speedup vs baseline: 1.5348x; 1.2057x over previous
"""LengthRegulator kernel for Trainium2 (Bass/Tile), 8-core data parallel.

Reference op, per batch row b:
    dur  = clamp(durations[b].astype(int32), min=0)               # [T]
    csum = cumsum(dur)                                            # [T] incl
    src[j] = searchsorted(csum, j, 'right')          j in [0, L)
    out[b, j] = x[b, src[j]] if j < csum[-1] else 0

Device algorithm (B=16 rows, 2 per core), bf16 data path:

  searchsorted via sorted-rank max-scan (NOT a counting histogram --
  dma_scatter_add duplicate indices race on HW and lose adds):
    csum is non-decreasing, so src[j] = max{t+1 : csum[t] <= j}.  For each
    "last token of its equal-csum group" (mask = dur[t+1] > 0, sentinel at
    t=T-1) scatter the VALUE t+1 into bin csum[t] -- bins are unique, so
    the scatter-add never races.  Bins are SBUF-parity-scatter encoded as
    sigma = (csum%256)*128 + csum//256: partition = csum//256 (<16 for
    valid j), slot = csum%256 -> parity tiles h_even/h_odd [128,128] f32.
    Tokens with csum >= 4096 land on partitions 16..28 (never read) --
    automatic dump; masked tokens are directed to partition 16.
    src[j] then = max(carry[j//256], running-max over slots of A0) where
    A0[q, s] = h[256q + s] (two strided DVE copies from h_even/h_odd) and
    carry[q] = #{t: csum[t] < 256q} (sorted => count == max-rank), computed
    off the critical path as is_ge-compare [16,16,64] -> X-reduce ->
    ones-matmul into PSUM [16,1].

  gather: idx block[a, b'] = src[16b' + a] == the i16 convert of the scan
    output directly (src16 [16,256] covers both 2048-row chunks), then
    3 partition-doubling DMAs replicate to the 8 SWDGE channel groups.
    dma_gather pulls 1KiB bf16 rows from x_pad (row T = zeros serves
    j >= total and src==T); gathered row i of chunk k is out row
    j = 2048k + 128*(i//128) + i%128.

  store: paged_writeback (V path, ncn=128, page_size=128, page-aligned
    static iota indices) scatter-writes the gather tile straight to the
    paged DRAM output -- modeled at ~257 descriptors per 2MiB chunk vs
    2048 for a plain DMA store, removing ~23us of serialized DMA time.
    Host-side de-paging: pages[r, pg, part, 1, :] -> out row 128*pg+part
    (pure layout slice, validated on HW).
"""

import numpy as np

B, T, D, L = 16, 1024, 512, 4096
NCORES = 8
RPC = B // NCORES  # batch rows per core
NPG = L // 128  # writeback pages per row (page_size=128)

_cache = {}


def _consts():
    # U16s[k, m] = 1 iff k < m  (strict upper: offs[m] = sum_{k<m} last[k])
    u16 = np.triu(np.ones((16, 16), np.float32), k=1)
    # thr[q, 64*p + f] = 256*p - 1 ; C = (thr >= csum) <=> csum < 256p
    thr = np.repeat((256.0 * np.arange(16) - 1.0), 64).astype(np.float32)
    thr = np.broadcast_to(thr, (16, 1024)).copy()
    # R[k, m] = 1 iff m%16 == k: one-hot partition-replication matmul
    rep = (np.arange(128)[None, :] % 16 == np.arange(16)[:, None]).astype(
        np.float32
    )
    # merged 16-row const block: [THR(1024) | U16(16) | R(128)]
    c16 = np.concatenate([thr, u16, rep], axis=1)
    # scatter value layout: desc i reads vals[i%128, i//128] and idx
    # block[i%16, i//16]; block[a, b] = sigma(t = 64a + b)  =>
    # vals[p, s] = t+1 with t = 64*(p%16) + 8*s + p//16
    p = np.arange(128)[:, None]
    s = np.arange(8)[None, :]
    vals = (64 * (p % 16) + 8 * s + p // 16 + 1).astype(np.float32)
    return c16, vals


def _build_nc(reps=1):
    import concourse.bacc as bacc
    import concourse.mybir as mybir
    import concourse.tile as tile
    from concourse import library_config
    from bass_rust import add_dep_helper

    f32 = mybir.dt.float32
    bf16 = mybir.dt.bfloat16
    i32 = mybir.dt.int32
    i16 = mybir.dt.int16
    Alu = mybir.AluOpType

    nc = bacc.Bacc("TRN2", target_bir_lowering=False)
    x_pad = nc.dram_tensor("x_pad", [RPC, T + 1, D], bf16, kind="ExternalInput")
    durs_in = nc.dram_tensor("durs", [RPC, 2, T], i32, kind="ExternalInput")
    pages = nc.dram_tensor(
        "pages", [RPC, NPG, 128, 2 * D], bf16, kind="ExternalOutput"
    )

    c16_h, vals_h = _consts()
    C16c = nc.inline_tensor(c16_h, name="C16c")
    VALc = nc.inline_tensor(vals_h, name="VALc")

    with tile.TileContext(nc) as tc:
        with (
            tc.tile_pool(name="const", bufs=1) as cpool,
            tc.tile_pool(name="small", bufs=2) as spool,
            tc.tile_pool(name="hist", bufs=2) as hpool,
            tc.tile_pool(name="idx", bufs=2) as ipool,
            tc.tile_pool(name="gath", bufs=3) as gpool,
            tc.tile_pool(name="ps", bufs=1, space="PSUM") as ppool,
        ):
            nc.gpsimd.load_library(library_config.attnmlp)

            # ---- inputs first (dur chain is the critical path) ----
            dur_t = spool.tile([16, RPC * 2 * 64], i32, tag="dur")
            nc.sync.dma_start(
                out=dur_t[:].rearrange("p (r c f) -> p r c f", r=RPC, c=2),
                in_=durs_in[:].rearrange("r c (p f) -> p r c f", p=16),
            )

            # ---- constants ----
            C16 = cpool.tile([16, 1024 + 16 + 128], f32)
            nc.scalar.dma_start(out=C16[:], in_=C16c[:])
            THR = C16[:, 0:1024]
            U16 = C16[:, 1024:1040]
            REP = C16[:, 1040:1168]
            VAL = cpool.tile([128, 8], f32)
            nc.scalar.dma_start(out=VAL[:], in_=VALc[:])
            ones16 = cpool.tile([16, 1], f32)
            nc.vector.memset(ones16[:], 1.0)
            # writeback idx consts: [ptr1(16) | ptr2(16) | pidx(16)] per chunk
            wbi = [
                cpool.tile([128, 48], i32, name=f"wbi{k}") for k in range(2)
            ]
            for k in range(2):
                nc.gpsimd.iota(
                    wbi[k][:, 0:16], pattern=[[1, 16]], base=16 * k,
                    channel_multiplier=0,
                )
                nc.vector.memset(wbi[k][:, 16:32], -1)
                nc.vector.memset(wbi[k][:, 32:48], 0)

            # durs tile layout: [:, (2r+c)*64 : +64] = row r, c=0 dur /
            # c=1 shifted dur (t = 64p + f)
            dur_f = spool.tile([16, RPC * 64], f32, tag="durf")
            m_i = spool.tile([16, RPC * 64], i32, tag="mi")
            for r in range(RPC):
                nc.vector.tensor_scalar(
                    dur_f[:, 64 * r : 64 * r + 64],
                    dur_t[:, 128 * r : 128 * r + 64], 0, None, Alu.max,
                )
                nc.vector.tensor_scalar(
                    m_i[:, 64 * r : 64 * r + 64],
                    dur_t[:, 128 * r + 64 : 128 * r + 128], 0, None,
                    Alu.is_gt,
                )

            # ---- phase A per row: csum -> sigma -> PE-replicated idx ----
            csums, scs, carrs = [], [], []
            for r in range(RPC):
                sl = slice(64 * r, 64 * r + 64)
                pref = spool.tile([16, 64], f32, tag=f"pref{r}")
                nc.vector.tensor_tensor_scan(
                    out=pref[:], data0=dur_f[:, sl], data1=dur_f[:, sl],
                    initial=0.0, op0=Alu.add, op1=Alu.bypass,
                )
                offs = ppool.tile([16, 1], f32, tag=f"offs{r}")
                nc.tensor.matmul(
                    out=offs[:], lhsT=U16, rhs=pref[:, 63:64],
                    start=True, stop=True,
                )
                csum = spool.tile([16, 64], f32, tag=f"csum{r}")
                nc.vector.tensor_tensor(
                    out=csum[:], in0=pref[:],
                    in1=offs[:].to_broadcast([16, 64]), op=Alu.add,
                )
                csums.append(csum)

                # sigma = (e&255)<<7 | e>>8, masked (non-last -> 16)
                e_i = spool.tile([16, 64], i32, tag=f"ei{r}")
                nc.vector.tensor_copy(out=e_i[:], in_=csum[:])
                sa = spool.tile([16, 64], i32, tag=f"sa{r}")
                nc.vector.tensor_scalar(
                    sa[:], e_i[:], 7, 32640, Alu.logical_shift_left,
                    Alu.bitwise_and,
                )
                hi = spool.tile([16, 64], i32, tag=f"hi{r}")
                nc.vector.tensor_scalar(
                    hi[:], e_i[:], 8, None, Alu.logical_shift_right
                )
                sg = spool.tile([16, 64], i32, tag=f"sg{r}")
                nc.vector.tensor_tensor(
                    out=sg[:], in0=sa[:], in1=hi[:], op=Alu.add
                )
                # select: sigma = m*(sg-16) + 16
                nc.vector.tensor_scalar(sg[:], sg[:], -16, None, Alu.add)
                nc.vector.tensor_tensor(
                    out=sg[:], in0=sg[:], in1=m_i[:, sl], op=Alu.mult
                )
                nc.vector.tensor_scalar(sg[:], sg[:], 16, None, Alu.add)
                # replicate across the 8 SWDGE channel groups via one-hot
                # matmul (each output is a single term -> exact)
                sgf = spool.tile([16, 64], f32, tag=f"sgf{r}")
                nc.vector.tensor_copy(out=sgf[:], in_=sg[:])
                scps = ppool.tile([128, 64], f32, tag=f"scps{r}")
                nc.tensor.matmul(
                    out=scps[:], lhsT=REP, rhs=sgf[:], start=True, stop=True
                )
                sc = ipool.tile([128, 64], i16, tag=f"sc{r}")
                nc.vector.tensor_copy(out=sc[:], in_=scps[:])
                scs.append(sc)

            # carry[q] = #{t: csum < 256q}: compare -> reduce -> matmul
            # (emitted after the sigma chain; not needed until post-readback)
            for r in range(RPC):
                C = spool.tile([16, 1024], f32, tag=f"C{r}")
                nc.vector.tensor_tensor(
                    out=C[:].rearrange("q (p f) -> q p f", f=64),
                    in0=THR.rearrange("q (p f) -> q p f", f=64),
                    in1=csums[r][:].unsqueeze(1).to_broadcast([16, 16, 64]),
                    op=Alu.is_ge,
                )
                Dm = spool.tile([16, 16], f32, tag=f"D{r}")
                nc.vector.tensor_reduce(
                    out=Dm[:],
                    in_=C[:].rearrange("q (p f) -> q p f", f=64),
                    axis=mybir.AxisListType.X, op=Alu.add,
                )
                carr = ppool.tile([16, 1], f32, tag=f"carr{r}")
                nc.tensor.matmul(
                    out=carr[:], lhsT=Dm[:], rhs=ones16[:],
                    start=True, stop=True,
                )
                carrs.append(carr)

            # scatters back-to-back on Pool so row1's can fire early
            hes, hos = [], []
            for r in range(RPC):
                h_e = hpool.tile([128, 128], f32, tag=f"he{r}")
                h_o = hpool.tile([128, 128], f32, tag=f"ho{r}")
                nc.vector.memset(h_e[:], 0.0)
                nc.vector.memset(h_o[:], 0.0)
                nc.gpsimd.dma_scatter_add(
                    out_ap=h_e[:],
                    in_ap=VAL[:].rearrange("p (s e) -> p s e", e=1),
                    idxs_ap=scs[r][:],
                    num_idxs=1024,
                    num_idxs_reg=1024,
                    elem_size=1,
                    single_packet=False,
                    sbuf_tokens_per_rank=128,
                    parity_reg=0,
                    out_ap_other=h_o[:],
                )
                hes.append(h_e)
                hos.append(h_o)

            # ---- phase B/C per row: scan -> src16 -> gather -> writeback
            for rep in range(reps):
                for r in range(RPC):
                    A0 = spool.tile([16, 256], f32, tag=f"A0{r}")
                    av = A0[:].rearrange("q (s two) -> q s two", two=2)
                    nc.vector.tensor_copy(
                        out=av[:, :, 0:1],
                        in_=hes[r][0:16, :].unsqueeze(2),
                    )
                    nc.vector.tensor_copy(
                        out=av[:, :, 1:2],
                        in_=hos[r][0:16, :].unsqueeze(2),
                    )
                    M = spool.tile([16, 256], f32, tag=f"M{r}")
                    nc.vector.tensor_tensor_scan(
                        out=M[:], data0=A0[:], data1=A0[:], initial=0.0,
                        op0=Alu.max, op1=Alu.bypass,
                    )
                    srcf = spool.tile([16, 256], f32, tag=f"srcf{r}")
                    nc.vector.tensor_tensor(
                        out=srcf[:], in0=M[:],
                        in1=carrs[r][:].to_broadcast([16, 256]), op=Alu.max,
                    )
                    # block[a, b] = srcf[a, 128k+b] = src[256a + 128k + b]
                    # (host de-paging applies the inverse permutation);
                    # channel-group replication via one-hot matmul (exact)
                    gips = ppool.tile([128, 256], f32, tag=f"gips{r}")
                    nc.tensor.matmul(
                        out=gips[:], lhsT=REP, rhs=srcf[:],
                        start=True, stop=True,
                    )
                    gis = []
                    for k in range(2):
                        gik = ipool.tile([128, 128], i16, tag=f"gi{r}{k}")
                        nc.vector.tensor_copy(
                            out=gik[:], in_=gips[:, 128 * k : 128 * k + 128]
                        )
                        gis.append(gik)

                    for k in range(2):
                        gt = gpool.tile([128, 16 * D], bf16, tag="gt")
                        nc.gpsimd.dma_gather(
                            out_ap=gt[:].rearrange("p (m e) -> p m e", e=D),
                            in_ap=x_pad[r],
                            idxs_ap=gis[k][:],
                            num_idxs=2048,
                            num_idxs_reg=2048,
                            elem_size=D,
                            single_packet=False,
                        )
                        wb = nc.gpsimd.paged_writeback(
                            out_ap=pages[r],
                            in_ap=gt[:].rearrange("p (m e) -> p m e", e=D),
                            idxs_ap=wbi[k][:],
                            batch=16,
                            ncn=128,
                            page_size=128,
                            d_head=D,
                            k_or_v="v",
                        )
                        fence = nc.gpsimd.nop(
                            nofuse=True, hint=f"wbf{r}{k}"
                        )
                        add_dep_helper(
                            fence.ins, wb.ins,
                            reason="kernel end waits writeback",
                        )
    nc.compile()
    return nc


def _get_nc(reps=1):
    if reps not in _cache:
        _cache[reps] = _build_nc(reps)
    return _cache[reps]


def kernel(x, durations, max_len):
    import ml_dtypes
    from concourse.bass_utils import run_bass_kernel_spmd

    x = np.asarray(x)
    durations = np.asarray(durations)
    assert x.shape == (B, T, D) and int(max_len) == L, (x.shape, max_len)

    dur32 = durations.astype(np.int32)  # truncating cast, same as reference
    # shifted durations for the "last of equal-csum group" mask; sentinel
    # makes t = T-1 always last.
    durn32 = np.concatenate(
        [dur32[:, 1:], np.full((B, 1), 8192, np.int32)], axis=1
    )
    durs = np.stack([dur32, durn32], axis=1)  # [B, 2, T]
    in_maps = []
    for core in range(NCORES):
        lo = core * RPC
        xp = np.zeros((RPC, T + 1, D), ml_dtypes.bfloat16)
        xp[:, :T, :] = x[lo : lo + RPC].astype(ml_dtypes.bfloat16)
        in_maps.append(
            {
                "x_pad": xp,
                "durs": np.ascontiguousarray(durs[lo : lo + RPC]),
            }
        )

    nc = _get_nc()
    res = run_bass_kernel_spmd(nc, in_maps, core_ids=list(range(NCORES)))
    # de-page permutation: gathered item i of chunk k holds out row
    # j = 256*(i%16) + 128k + i//16; it lands in page 16k + (i//128) at
    # position u = i%128. Inverting: for out row j,
    #   k = (j//128) % 2, b = (j%128)//8, u = 16*(j%8) + j//256.
    jj = np.arange(L)
    pgi = 16 * ((jj // 128) % 2) + (jj % 128) // 8
    ui = 16 * (jj % 8) + jj // 256
    outs = []
    for c in range(NCORES):
        pg = np.asarray(res.results[c]["pages"])  # [RPC, NPG, 128, 1024]
        pg5 = pg.reshape(RPC, NPG, 128, 2, D)
        rows = pg5[:, pgi, ui, 1, :]  # [RPC, L, D]
        outs.append(rows.astype(np.float32))
    return np.concatenate(outs, axis=0).reshape(B, L, D)


# revision 23
# speedup vs baseline: 1.8685x; 1.2174x over previous
"""LengthRegulator kernel for Trainium2 (Bass/Tile), 8-core data parallel.

Reference op, per batch row b:
    dur  = clamp(durations[b].astype(int32), min=0)               # [T]
    csum = cumsum(dur)                                            # [T] incl
    src[j] = searchsorted(csum, j, 'right')          j in [0, L)
    out[b, j] = x[b, src[j]] if j < csum[-1] else 0

Device algorithm (B=16 rows, 2 per core), bf16 data path:

  searchsorted via sorted-rank max-scan (NOT a counting histogram --
  dma_scatter_add duplicate indices race on HW and lose adds):
    csum is non-decreasing, so src[j] = max{t+1 : csum[t] <= j}.  For each
    "last token of its equal-csum group" (mask = dur[t+1] > 0, sentinel at
    t=T-1) scatter the VALUE t+1 into bin csum[t] -- bins are unique, so
    the scatter-add never races.  Bins are SBUF-parity-scatter encoded as
    sigma = (csum%256)*128 + csum//256: partition = csum//256 (<16 for
    valid j), slot = csum%256 -> parity tiles h_even/h_odd [128,128] f32.
    Tokens with csum >= 4096 land on partitions 16..28 (never read) --
    automatic dump; masked tokens are directed to partition 16.
    src[j] then = max(carry[j//256], running-max over slots of A0) where
    A0[q, s] = h[256q + s] (two strided DVE copies from h_even/h_odd) and
    carry[q] = #{t: csum[t] < 256q} (sorted => count == max-rank), computed
    off the critical path as is_ge-compare [16,16,64] -> X-reduce ->
    ones-matmul into PSUM [16,1].

  gather: idx block[a, b'] = src[16b' + a] == the i16 convert of the scan
    output directly (src16 [16,256] covers both 2048-row chunks), then
    3 partition-doubling DMAs replicate to the 8 SWDGE channel groups.
    dma_gather pulls 1KiB bf16 rows from x_pad (row T = zeros serves
    j >= total and src==T); gathered row i of chunk k is out row
    j = 2048k + 128*(i//128) + i%128.

  store: paged_writeback (V path, ncn=128, page_size=128, page-aligned
    static iota indices) scatter-writes the gather tile straight to the
    paged DRAM output -- modeled at ~257 descriptors per 2MiB chunk vs
    2048 for a plain DMA store, removing ~23us of serialized DMA time.
    Host-side de-paging: pages[r, pg, part, 1, :] -> out row 128*pg+part
    (pure layout slice, validated on HW).
"""

import numpy as np

B, T, D, L = 16, 1024, 512, 4096
NCORES = 8
RPC = B // NCORES  # batch rows per core
NPG = L // 128  # writeback pages per row (page_size=128)

_cache = {}


def _consts():
    # U16s[k, m] = 1 iff k < m  (strict upper: offs[m] = sum_{k<m} last[k])
    u16 = np.triu(np.ones((16, 16), np.float32), k=1)
    # thr[q, 64*p + f] = 256*p - 1 ; C = (thr >= csum) <=> csum < 256p
    thr = np.repeat((256.0 * np.arange(16) - 1.0), 64).astype(np.float32)
    thr = np.broadcast_to(thr, (16, 1024)).copy()
    # R[k, m] = 1 iff m%16 == k: one-hot partition-replication matmul
    rep = (np.arange(128)[None, :] % 16 == np.arange(16)[:, None]).astype(
        np.float32
    )
    # merged 16-row const block: [U16(16) | R(128)]
    c16 = np.concatenate([u16, rep], axis=1)
    # PC[q, 64*p + f] = p (bf16-exact); carry compare runs in bf16 2x mode
    import ml_dtypes
    pc = np.repeat(np.arange(16, dtype=np.float32), 64)
    pc = np.broadcast_to(pc, (16, 1024)).astype(ml_dtypes.bfloat16)
    # scatter value layout: desc i reads vals[i%128, i//128] and idx
    # block[i%16, i//16]; block[a, b] = sigma(t = 64a + b)  =>
    # vals[p, s] = t+1 with t = 64*(p%16) + 8*s + p//16
    p = np.arange(128)[:, None]
    s = np.arange(8)[None, :]
    vals = (64 * (p % 16) + 8 * s + p // 16 + 1).astype(np.float32)
    return c16, pc, vals


def _build_nc(reps=1):
    import concourse.bacc as bacc
    import concourse.mybir as mybir
    import concourse.tile as tile
    from concourse import library_config
    from bass_rust import add_dep_helper

    f32 = mybir.dt.float32
    bf16 = mybir.dt.bfloat16
    i32 = mybir.dt.int32
    i16 = mybir.dt.int16
    Alu = mybir.AluOpType

    nc = bacc.Bacc(
        "TRN2", target_bir_lowering=False, dynamic_dma_scratch_size=65536
    )
    x_pad = nc.dram_tensor("x_pad", [RPC, T + 1, D], bf16, kind="ExternalInput")
    durs_in = nc.dram_tensor("durs", [RPC, 2, T], i32, kind="ExternalInput")
    pages = nc.dram_tensor(
        "pages", [RPC, NPG, 128, 2 * D], bf16, kind="ExternalOutput"
    )

    c16_h, pc_h, vals_h = _consts()
    C16c = nc.inline_tensor(c16_h, name="C16c")
    PCc = nc.inline_tensor(pc_h, name="PCc")
    VALc = nc.inline_tensor(vals_h, name="VALc")

    with tile.TileContext(nc) as tc:
        with (
            tc.tile_pool(name="const", bufs=1) as cpool,
            tc.tile_pool(name="small", bufs=2) as spool,
            tc.tile_pool(name="hist", bufs=2) as hpool,
            tc.tile_pool(name="idx", bufs=2) as ipool,
            tc.tile_pool(name="gath", bufs=3) as gpool,
            tc.tile_pool(name="ps", bufs=1, space="PSUM") as ppool,
        ):
            nc.gpsimd.load_library(library_config.attnmlp)

            # ---- inputs first (dur chain is the critical path) ----
            dur_t = spool.tile([16, RPC * 2 * 64], i32, tag="dur")
            nc.sync.dma_start(
                out=dur_t[:].rearrange("p (r c f) -> p r c f", r=RPC, c=2),
                in_=durs_in[:].rearrange("r c (p f) -> p r c f", p=16),
            )

            # ---- constants ----
            C16 = cpool.tile([16, 16 + 128], f32)
            nc.scalar.dma_start(out=C16[:], in_=C16c[:])
            U16 = C16[:, 0:16]
            REP = C16[:, 16:144]
            PC = cpool.tile([16, 1024], bf16)
            nc.scalar.dma_start(out=PC[:], in_=PCc[:])
            VAL = cpool.tile([128, 8], f32)
            nc.scalar.dma_start(out=VAL[:], in_=VALc[:])
            ones16b = cpool.tile([16, 1], bf16)
            nc.vector.memset(ones16b[:], 1.0)
            # writeback idx consts: [ptr1(16) | ptr2(16) | pidx(16)] per chunk
            wbi = [
                cpool.tile([128, 48], i32, name=f"wbi{k}") for k in range(2)
            ]
            for k in range(2):
                nc.gpsimd.iota(
                    wbi[k][:, 0:16], pattern=[[1, 16]], base=16 * k,
                    channel_multiplier=0,
                )
                nc.vector.memset(wbi[k][:, 16:32], -1)
                nc.vector.memset(wbi[k][:, 32:48], 0)

            # durs tile layout: [:, (2r+c)*64 : +64] = row r, c=0 dur /
            # c=1 shifted dur (t = 64p + f)
            dur_f = spool.tile([16, RPC * 64], f32, tag="durf")
            m_i = spool.tile([16, RPC * 64], i32, tag="mi")
            for r in range(RPC):
                nc.vector.tensor_scalar(
                    dur_f[:, 64 * r : 64 * r + 64],
                    dur_t[:, 128 * r : 128 * r + 64], 0, None, Alu.max,
                )
                nc.vector.tensor_scalar(
                    m_i[:, 64 * r : 64 * r + 64],
                    dur_t[:, 128 * r + 64 : 128 * r + 128], 0, None,
                    Alu.is_gt,
                )

            # ---- phase A: csum per row, then both-rows sigma in one go --
            csum2 = spool.tile([16, RPC * 64], f32, tag="csum2")
            for r in range(RPC):
                sl = slice(64 * r, 64 * r + 64)
                pref = spool.tile([16, 64], f32, tag=f"pref{r}")
                nc.vector.tensor_tensor_scan(
                    out=pref[:], data0=dur_f[:, sl], data1=dur_f[:, sl],
                    initial=0.0, op0=Alu.add, op1=Alu.bypass,
                )
                offs = ppool.tile([16, 1], f32, tag="offs")
                nc.tensor.matmul(
                    out=offs[:], lhsT=U16, rhs=pref[:, 63:64],
                    start=True, stop=True,
                )
                nc.vector.tensor_tensor(
                    out=csum2[:, sl], in0=pref[:],
                    in1=offs[:].to_broadcast([16, 64]), op=Alu.add,
                )

            # sigma = (e&255)<<7 | e>>8, masked (non-last -> 16); per row so
            # row 0's scatter does not wait on row 1's csum
            scs, sc_cvts, his = [], [], []
            for r in range(RPC):
                sl = slice(64 * r, 64 * r + 64)
                e_i = spool.tile([16, 64], i32, tag=f"ei{r}")
                nc.vector.tensor_copy(out=e_i[:], in_=csum2[:, sl])
                sa = spool.tile([16, 64], i32, tag=f"sa{r}")
                nc.vector.tensor_scalar(
                    sa[:], e_i[:], 7, 32640, Alu.logical_shift_left,
                    Alu.bitwise_and,
                )
                hi = spool.tile([16, 64], i32, tag=f"hi{r}")
                nc.vector.tensor_scalar(
                    hi[:], e_i[:], 8, None, Alu.logical_shift_right
                )
                his.append(hi)
                sg = spool.tile([16, 64], i32, tag=f"sg{r}")
                nc.vector.tensor_tensor(
                    out=sg[:], in0=sa[:], in1=hi[:], op=Alu.add
                )
                nc.vector.tensor_scalar(sg[:], sg[:], -16, None, Alu.add)
                nc.vector.tensor_tensor(
                    out=sg[:], in0=sg[:], in1=m_i[:, sl], op=Alu.mult
                )
                nc.vector.tensor_scalar(sg[:], sg[:], 16, None, Alu.add)
                sgf = spool.tile([16, 64], f32, tag=f"sgf{r}")
                nc.vector.tensor_copy(out=sgf[:], in_=sg[:])
                # replicate across the 8 SWDGE channel groups via one-hot
                # matmul (each output is a single term -> exact)
                scps = ppool.tile([128, 64], f32, tag="scps")
                nc.tensor.matmul(
                    out=scps[:], lhsT=REP, rhs=sgf[:], start=True, stop=True
                )
                sc = ipool.tile([128, 64], i16, tag=f"sc{r}")
                cv = nc.vector.tensor_copy(out=sc[:], in_=scps[:])
                scs.append(sc)
                sc_cvts.append(cv)
            carrs = []

            # scatters back-to-back on Pool so row1's can fire early
            hes, hos = [], []
            for r in range(RPC):
                h_e = hpool.tile([128, 128], f32, tag=f"he{r}")
                h_o = hpool.tile([128, 128], f32, tag=f"ho{r}")
                nc.vector.memset(h_e[:], 0.0)
                nc.vector.memset(h_o[:], 0.0)
                nc.gpsimd.dma_scatter_add(
                    out_ap=h_e[:],
                    in_ap=VAL[:].rearrange("p (s e) -> p s e", e=1),
                    idxs_ap=scs[r][:],
                    num_idxs=1024,
                    num_idxs_reg=1024,
                    elem_size=1,
                    single_packet=False,
                    sbuf_tokens_per_rank=128,
                    parity_reg=0,
                    out_ap_other=h_o[:],
                )
                hes.append(h_e)
                hos.append(h_o)

            # carry[q] = #{t: csum < 256q} = #{t: csum>>8 < q}: bf16 2x-mode
            # compare + reduce (hi <= 28 and q <= 15 are bf16-exact), then a
            # ones-matmul sums over token partitions into PSUM [16, 1].
            for r in range(RPC):
                hb = spool.tile([16, 64], bf16, tag=f"hb{r}")
                cvh = nc.vector.tensor_copy(out=hb[:], in_=his[r][:])
                add_dep_helper(
                    cvh.ins, sc_cvts[-1].ins,
                    reason="defer carry ops past scatter-idx converts",
                )
                C = spool.tile([16, 1024], bf16, tag=f"C{r}")
                nc.vector.tensor_tensor(
                    out=C[:].rearrange("q (p f) -> q p f", f=64),
                    in0=PC[:].rearrange("q (p f) -> q p f", f=64),
                    in1=hb[:].unsqueeze(1).to_broadcast([16, 16, 64]),
                    op=Alu.is_gt,
                )
                Dm = spool.tile([16, 16], bf16, tag=f"D{r}")
                with nc.allow_low_precision(
                    reason="counts <= 64 are bf16-exact"
                ):
                    red = nc.vector.tensor_reduce(
                        out=Dm[:],
                        in_=C[:].rearrange("q (p f) -> q p f", f=64),
                        axis=mybir.AxisListType.X, op=Alu.add,
                    )
                carr = ppool.tile([16, 1], f32, tag=f"carr{r}")
                nc.tensor.matmul(
                    out=carr[:], lhsT=Dm[:], rhs=ones16b[:],
                    start=True, stop=True,
                )
                carrs.append(carr)
                last_red = red

            # ---- phase B/C per row: scan -> src16 -> gather -> writeback
            prev_scan = None
            for rep in range(reps):
                for r in range(RPC):
                    A0 = spool.tile([16, 256], f32, tag=f"A0{r}")
                    av = A0[:].rearrange("q (s two) -> q s two", two=2)
                    cp0 = nc.vector.tensor_copy(
                        out=av[:, :, 0:1],
                        in_=hes[r][0:16, :].unsqueeze(2),
                    )
                    if r == 0 and rep == 0:
                        # reduces fill the DVE idle gap before the scatter
                        # readback; keep the A0 copies behind them
                        add_dep_helper(
                            cp0.ins, last_red.ins,
                            reason="A0 copies after carry reduces",
                        )
                    if prev_scan is not None:
                        add_dep_helper(
                            cp0.ins, prev_scan.ins,
                            reason="keep row r A0 copies behind row r-1 scan",
                        )
                    cp1 = nc.vector.tensor_copy(
                        out=av[:, :, 1:2],
                        in_=hos[r][0:16, :].unsqueeze(2),
                    )
                    if r == 0 and rep == 0:
                        add_dep_helper(
                            cp1.ins, last_red.ins,
                            reason="A0 copies after carry reduces",
                        )
                    if prev_scan is not None:
                        add_dep_helper(
                            cp1.ins, prev_scan.ins,
                            reason="keep row r A0 copies behind row r-1 scan",
                        )
                    M = spool.tile([16, 256], f32, tag=f"M{r}")
                    prev_scan = nc.vector.tensor_tensor_scan(
                        out=M[:], data0=A0[:], data1=A0[:], initial=0.0,
                        op0=Alu.max, op1=Alu.bypass,
                    )
                    srcf = spool.tile([16, 256], f32, tag=f"srcf{r}")
                    nc.vector.tensor_tensor(
                        out=srcf[:], in0=M[:],
                        in1=carrs[r][:].to_broadcast([16, 256]), op=Alu.max,
                    )
                    # block[a, b] = srcf[a, 128k+b] = src[256a + 128k + b]
                    # (host de-paging applies the inverse permutation);
                    # channel-group replication via one-hot matmul (exact),
                    # split per chunk so gather k=0 starts sooner
                    gis = []
                    for k in range(2):
                        gipk = ppool.tile([128, 128], f32, tag=f"gip{k}")
                        nc.tensor.matmul(
                            out=gipk[:], lhsT=REP,
                            rhs=srcf[:, 128 * k : 128 * k + 128],
                            start=True, stop=True,
                        )
                        gik = ipool.tile([128, 128], i16, tag=f"gi{r}{k}")
                        nc.vector.tensor_copy(out=gik[:], in_=gipk[:])
                        gis.append(gik)

                    gts = []
                    for k in range(2):
                        gt = gpool.tile([128, 16 * D], bf16, tag="gt")
                        nc.gpsimd.dma_gather(
                            out_ap=gt[:].rearrange("p (m e) -> p m e", e=D),
                            in_ap=x_pad[r],
                            idxs_ap=gis[k][:],
                            num_idxs=2048,
                            num_idxs_reg=2048,
                            elem_size=D,
                            single_packet=False,
                        )
                        gts.append(gt)
                    for k in range(2):
                        wb = nc.gpsimd.paged_writeback(
                            out_ap=pages[r],
                            in_ap=gts[k][:].rearrange("p (m e) -> p m e", e=D),
                            idxs_ap=wbi[k][:],
                            batch=16,
                            ncn=128,
                            page_size=128,
                            d_head=D,
                            k_or_v="v",
                        )
                        fence = nc.gpsimd.nop(nofuse=True, hint=f"wbf{r}{k}")
                        add_dep_helper(
                            fence.ins, wb.ins,
                            reason="kernel end waits writeback",
                        )
    nc.compile()
    return nc


def _get_nc(reps=1):
    if reps not in _cache:
        _cache[reps] = _build_nc(reps)
    return _cache[reps]


def kernel(x, durations, max_len):
    import ml_dtypes
    from concourse.bass_utils import run_bass_kernel_spmd

    x = np.asarray(x)
    durations = np.asarray(durations)
    assert x.shape == (B, T, D) and int(max_len) == L, (x.shape, max_len)

    dur32 = durations.astype(np.int32)  # truncating cast, same as reference
    # shifted durations for the "last of equal-csum group" mask; sentinel
    # makes t = T-1 always last.
    durn32 = np.concatenate(
        [dur32[:, 1:], np.full((B, 1), 8192, np.int32)], axis=1
    )
    durs = np.stack([dur32, durn32], axis=1)  # [B, 2, T]
    in_maps = []
    for core in range(NCORES):
        lo = core * RPC
        xp = np.zeros((RPC, T + 1, D), ml_dtypes.bfloat16)
        xp[:, :T, :] = x[lo : lo + RPC].astype(ml_dtypes.bfloat16)
        in_maps.append(
            {
                "x_pad": xp,
                "durs": np.ascontiguousarray(durs[lo : lo + RPC]),
            }
        )

    nc = _get_nc()
    res = run_bass_kernel_spmd(nc, in_maps, core_ids=list(range(NCORES)))
    # de-page permutation: gathered item i of chunk k holds out row
    # j = 256*(i%16) + 128k + i//16; it lands in page 16k + (i//128) at
    # position u = i%128. Inverting: for out row j,
    #   k = (j//128) % 2, b = (j%128)//8, u = 16*(j%8) + j//256.
    jj = np.arange(L)
    pgi = 16 * ((jj // 128) % 2) + (jj % 128) // 8
    ui = 16 * (jj % 8) + jj // 256
    outs = []
    for c in range(NCORES):
        pg = np.asarray(res.results[c]["pages"])  # [RPC, NPG, 128, 1024]
        pg5 = pg.reshape(RPC, NPG, 128, 2, D)
        rows = pg5[:, pgi, ui, 1, :]  # [RPC, L, D]
        outs.append(rows.astype(np.float32))
    return np.concatenate(outs, axis=0).reshape(B, L, D)
